# revision 18
# baseline (speedup 1.0000x reference)
"""GAT (nn_GAT_29523605193094) Trainium2 kernel.

The reference keeps the source bug ``src, dst = edges[0], edges[0]``, so the
adjacency matrix is purely diagonal: adj[i, i] = (i appears in edges[0]).
After the -inf masking, row i of the [N, N, H] score tensor has exactly one
finite entry (j = i) when node i is covered, so softmax over axis=1 yields
exactly 1.0 at (i, i) and 0.0 elsewhere, and the output row is exactly
h[i] = (X @ W)[i].  Rows for uncovered nodes are all -inf -> softmax is NaN
-> the output row is NaN.  Both cases are reproduced here:

    out = X @ W            (on 8 NeuronCores, row-sharded)
    out[~covered] = NaN    (host-side mask from edges[0])

The device work is a row-sharded [4096, 512] @ [512, 256] matmul, run in
bf16 (inputs cast on host; fp32 PSUM accumulation).  The fp32 harness
tolerance is 2e-2 relative to absmax(expected); bf16 lands at ~4.2e-3.

Per-core schedule notes (calibrated against NTFF profiles):
- W and the X^T shard are packed on host into ONE partition-major dram
  tensor, interleaved in matmul consumption order, so the first DMA call
  carries exactly what the first matmul needs (W c0-block k0 + X^T k0).
  Each HWDGE call costs ~0.65us issue + ~0.43us inter-call gap and its
  completion sem fires ~0.55us after the last byte, so call boundaries
  are placed to feed the matmul chain just in time.
- Calls split across both HWDGE rings (aggregate ~230-300GB/s; the second
  ring starts ~1us after the first): sync ring carries k0, k1, then k2-3
  as one big-line call; the scalar ring carries the small W c1 half early.
  Matmuls interleave c0/c1 per k-block so each arriving block feeds two
  matmuls and the chain stays compute-bound.
- PE warm-up via dummy matmuls is a NET LOSS here: any Tensor-engine
  activity during the input-DMA window delays DMA engine E79 by
  ~125ns/instruction, which delays the input completion sems by 2us+.
  The matmuls run cold (427ns each at 1.2GHz) by design.
- W k-chunks [128, 128] are PE-stationary; X^T chunks [128, 512] stream as
  the moving operand, accumulating into 2 PSUM banks (c = output column
  block, c-outer order).  The output leaves the device transposed
  ([OUT, RB] = h^T); the tail output is cast+written in halves across both
  rings so the HBM write receipts overlap.
"""

import numpy as np
import ml_dtypes

N = 4096
IN = 512
OUT = 256
NCORES = 8
RB = N // NCORES  # 512 rows per core
P = 128
KT = IN // P      # 4 contraction chunks
CT = OUT // P     # 2 output column blocks
BLK = P + RB      # 640: one k-block = W c0-chunk + X^T chunk
XIN = KT * BLK + KT * P  # 3072: 4 k-blocks + W c1 half

_state = {}

# test.py reads this after a traced call for the HW exec time.
LAST_RESULTS = None


def _build():
    import concourse.mybir as mybir
    import concourse.tile as tile
    from concourse import bacc

    nc = bacc.Bacc(
        "TRN2",
        target_bir_lowering=False,
        debug=False,
        num_devices=NCORES,
    )
    f32 = mybir.dt.float32
    bf16 = mybir.dt.bfloat16
    xin = nc.dram_tensor("xin", [P, XIN], bf16, kind="ExternalInput")
    out = nc.dram_tensor("out", [OUT, RB], bf16, kind="ExternalOutput")  # h^T

    with tile.TileContext(nc) as tc:
        with (
            tc.tile_pool(name="ins", bufs=1) as in_pool,
            tc.tile_pool(name="outs", bufs=2) as out_pool,
            tc.tile_pool(name="ps", bufs=2, space="PSUM") as psum_pool,
        ):
            xin_t = in_pool.tile([P, XIN], bf16)

            # Input calls: the sync ring (faster, earlier) carries the k0
            # and k1 blocks then k2-3 as one big-line call; the scalar ring
            # delivers the small W c1 half early.  The matmul order below
            # is interleaved so each input is needed at or after its
            # measured arrival time.
            WC1 = KT * BLK  # 2560: offset of the W c1 half
            nc.sync.dma_start(xin_t[:, 0:BLK], xin[:, 0:BLK])                  # k0
            nc.scalar.dma_start(xin_t[:, WC1:XIN], xin[:, WC1:XIN])             # W c1
            nc.sync.dma_start(xin_t[:, BLK : 2 * BLK], xin[:, BLK : 2 * BLK])   # k1
            nc.sync.dma_start(xin_t[:, 2 * BLK : WC1], xin[:, 2 * BLK : WC1])   # k2,k3

            ps0 = psum_pool.tile([P, RB], f32, name="ps0", tag="ps")
            ps1 = psum_pool.tile([P, RB], f32, name="ps1", tag="ps")
            # (c, k) interleaved so the last-arriving inputs (w_c1, k3) are
            # consumed last; start/stop mark each PSUM accumulation group.
            order = [(0, 0), (1, 0), (0, 1), (1, 1), (0, 2), (0, 3), (1, 2), (1, 3)]
            for c, k in order:
                if c == 0:
                    lhsT = xin_t[:, k * BLK : k * BLK + P]
                else:
                    lhsT = xin_t[:, WC1 + k * P : WC1 + (k + 1) * P]
                nc.tensor.matmul(
                    ps0[:] if c == 0 else ps1[:],
                    lhsT,
                    xin_t[:, k * BLK + P : (k + 1) * BLK],
                    start=(k == 0),
                    stop=(k == KT - 1),
                )
                if (c, k) == (0, 3):
                    # c0 complete: cast + write while c1's matmuls run.
                    ob0 = out_pool.tile([P, RB], bf16)
                    nc.vector.tensor_copy(ob0[:], ps0[:])
                    nc.scalar.dma_start(out[0:P, :], ob0[:])
            # Tail: cast c1 in halves so the first half's DMA (sync ring)
            # issues while the second half casts; write receipts overlap.
            HB = RB // 2
            ob1 = out_pool.tile([P, RB], bf16)
            nc.vector.tensor_copy(ob1[:, 0:HB], ps1[:, 0:HB])
            nc.sync.dma_start(out[P : 2 * P, 0:HB], ob1[:, 0:HB])
            nc.vector.tensor_copy(ob1[:, HB:RB], ps1[:, HB:RB])
            nc.scalar.dma_start(out[P : 2 * P, HB:RB], ob1[:, HB:RB])

    nc.compile()
    return nc


def kernel(X, edges, W, A):
    global LAST_RESULTS
    from concourse.bass_utils import run_bass_kernel_spmd

    X = np.asarray(X, dtype=np.float32)
    W = np.asarray(W, dtype=np.float32)
    edges = np.asarray(edges)

    if "nc" not in _state:
        _state["nc"] = _build()
    nc = _state["nc"]

    # Pack W + X^T shard into one partition-major tensor, interleaved in
    # consumption order: per partition p the layout is
    #   [W[k0,p,c0] | XT[k0,p,:] | ... | W[k3,p,c0] | XT[k3,p,:] | W[k*,p,c1]]
    XT = np.ascontiguousarray(X.T).astype(ml_dtypes.bfloat16)  # [IN, N]
    Wb = W.astype(ml_dtypes.bfloat16).reshape(KT, P, CT, P)  # [k, p, c, j]
    in_maps = []
    for core in range(NCORES):
        shard = XT[:, core * RB : (core + 1) * RB].reshape(KT, P, RB)
        xin = np.empty((P, XIN), dtype=ml_dtypes.bfloat16)
        for k in range(KT):
            xin[:, k * BLK : k * BLK + P] = Wb[k, :, 0, :]
            xin[:, k * BLK + P : (k + 1) * BLK] = shard[k]
            xin[:, KT * BLK + k * P : KT * BLK + (k + 1) * P] = Wb[k, :, 1, :]
        in_maps.append({"xin": xin})
    # The device occasionally reports a transient NRT_EXEC_UNIT_UNRECOVERABLE
    # on an otherwise-good kernel; retry before giving up.
    last_exc = None
    for _attempt in range(3):
        try:
            res = run_bass_kernel_spmd(nc, in_maps, core_ids=list(range(NCORES)))
            break
        except Exception as exc:  # noqa: BLE001
            last_exc = exc
            import time

            time.sleep(2.0)
    else:
        raise last_exc
    LAST_RESULTS = res
    # Per-core output is h_shard^T [OUT, RB]; stitch columns then transpose.
    out_t = np.concatenate(
        [np.asarray(res.results[c]["out"]) for c in range(NCORES)], axis=1
    )  # [OUT, N]
    out = out_t.T.astype(np.float32)

    # Reference semantics: nodes absent from edges[0] have an all -inf score
    # row; softmax of that is NaN, which propagates to the output row.
    covered = np.zeros(N, dtype=bool)
    covered[edges[0]] = True
    if not covered.all():
        out[~covered] = np.nan
    return out
